# revision 15
# baseline (speedup 1.0000x reference)
"""BiLSTM Enc-Dec + CRF NLL loss on 2 Trainium2 cores (SPMD, fwd/bwd split).

Strategy
--------
Batch=1 sequence, T=2048. The four BiLSTM scans (enc L0 -> enc L1 -> dec L0
-> dec L1) are inherently sequential in time; within each layer the forward
and backward direction are independent. So: core 0 runs all forward-direction
scans, core 1 runs all backward-direction scans, with one identical (symmetric)
SPMD program. Direction asymmetry is absorbed into per-core *data*:
  - core 1 receives the embedding sequence time-reversed, so its "forward"
    scan IS the backward scan;
  - per-core weight tensors are the own-direction slices, with gate rows
    permuted to [i, f, o, g];
  - cross-core exchanges (layer outputs, final states) use AllGather on
    internal DRAM bounce buffers.
Input projections x @ W_ih^T for a whole layer are big parallel matmuls
computed once per stage into DRAM (bf16), streamed into SBUF in windows
during the scan. The recurrent matvec h @ W_hh^T runs on the tensor engine
as 64 fp8 [128x128] weight-stationary matmuls per step against a small
STATIC h tile (static access patterns keep the PE pipelined at ~55-70ns
per LDWEIGHTS+MATMUL pair); the per-step xp slice is injected into PSUM
by one identity matmul so the sigmoid reads PSUM directly.

The CRF forward pass runs in the linear domain: alpha' = exp(trans) @ alpha
(a single stationary 48x48 matmul per step) times exp(feats_t), renormalized
each step by its sum; log of the normalizer is accumulated on the host in
float64. The CRF score term (tag path score) is computed on the host from
the device-computed feats.
"""

import sys

sys.path.insert(0, "/opt/trn_rl_repo")

import numpy as np
import ml_dtypes

import concourse.bacc as bacc
import concourse.mybir as mybir
from concourse.bass import ds
from concourse.tile import TileContext
from concourse.bass_utils import run_bass_kernel_spmd

# problem dims (hardcoded per spec)
T = 2048
ELMO = 1024
H = 512
POS = 64
K = 48
S = 50
L = 2
NEG = -10000.0
START_IDX, END_IDX = 0, 1

Din0 = ELMO + POS  # 1088
K0C = 9  # ceil(1088/128) k-tiles for layer-0 input (padded to 1152)
HC = 4  # h chunks of 128
G = 4 * H  # 2048 gates
GC = 16  # gate chunks of 128
U = 32  # scan steps unrolled per hardware-loop iteration
CH = 512  # scan steps per xp SBUF window
UCRF = 16

bf16 = mybir.dt.bfloat16
fp8 = mybir.dt.float8e4
f32 = mybir.dt.float32
AF = mybir.ActivationFunctionType
ALU = mybir.AluOpType
np_fp8 = ml_dtypes.float8_e4m3

_CACHE = {}


# ----------------------------------------------------------------------------
# host-side weight preparation
# ----------------------------------------------------------------------------

def _perm_gates(a, gscale=1.0):
    """reorder gate rows [i,f,g,o] -> [i,f,o,g] along axis 0 (size 4H)."""
    return np.concatenate(
        [a[0:H], a[H : 2 * H], a[3 * H : 4 * H], gscale * a[2 * H : 3 * H]], 0
    )


def _tile_kT(wT, nk):
    """[Ktot, M] -> [128, nk*M] with col kc*M + m = wT[kc*128 + p, m]."""
    Ktot, M = wT.shape
    assert Ktot == nk * 128
    return np.ascontiguousarray(wT.reshape(nk, 128, M).transpose(1, 0, 2).reshape(128, nk * M))


def _prep_core(inputs, d):
    """Build the per-core input map for direction d (0=fwd core, 1=bwd core)."""
    f = np.float32
    ins = {}
    sentence = inputs["sentence"].astype(f)
    pos_emb = inputs["pos_emb"].astype(f)
    speech = inputs["speech_tags"].astype(np.int64)
    embeds = np.concatenate([sentence, pos_emb[speech]], axis=1)  # (T, 1088)
    if d == 1:
        embeds = embeds[::-1]
    embT = np.zeros((K0C * 128, T), f)
    embT[:Din0] = embeds.T
    ins["embT"] = _tile_kT(embT, K0C).astype(ml_dtypes.bfloat16)
    ins["ident"] = np.eye(128, dtype=ml_dtypes.bfloat16)

    for model in ("enc", "dec"):
        for layer in (0, 1):
            whh = _perm_gates(inputs[f"{model}_w_hh{layer}"][d].astype(f))
            ins[f"whhT_{model}{layer}"] = _tile_kT(
                np.ascontiguousarray(whh.T), HC
            ).astype(np_fp8)
            b = _perm_gates(
                (inputs[f"{model}_b_ih{layer}"][d] + inputs[f"{model}_b_hh{layer}"][d]).astype(f)
            )
            ins[f"bias_{model}{layer}"] = np.ascontiguousarray(
                b.reshape(GC, 128).T
            ).astype(f)  # [128,16] col mc
        wih0 = _perm_gates(inputs[f"{model}_w_ih0"][d].astype(f))  # (2048, 1088)
        w0T = np.zeros((K0C * 128, G), f)
        w0T[:Din0] = wih0.T
        ins[f"wih0T_{model}"] = _tile_kT(w0T, K0C).astype(ml_dtypes.bfloat16)
        wih1 = _perm_gates(inputs[f"{model}_w_ih1"][d].astype(f))  # (2048, 1024)
        own = wih1[:, d * H : (d + 1) * H]
        peer = wih1[:, (1 - d) * H : (2 - d) * H]
        ins[f"wih1T_own_{model}"] = _tile_kT(np.ascontiguousarray(own.T), HC).astype(
            ml_dtypes.bfloat16
        )
        ins[f"wih1T_peer_{model}"] = _tile_kT(np.ascontiguousarray(peer.T), HC).astype(
            ml_dtypes.bfloat16
        )

    # e2h/e2c: rows = own dec init states, cols permuted to AllGather order.
    col_perm = np.concatenate(
        [
            np.arange(0, H),  # l0f
            np.arange(2 * H, 3 * H),  # l1f
            np.arange(H, 2 * H),  # l0b
            np.arange(3 * H, 4 * H),  # l1b
        ]
    )
    row_sel = np.concatenate([np.arange(d * H, (d + 1) * H), np.arange((2 + d) * H, (3 + d) * H)])
    for nm in ("e2h", "e2c"):
        w = inputs[f"{nm}_w"].astype(f)[row_sel][:, col_perm]  # (1024, 2048)
        ins[f"{nm}T"] = _tile_kT(np.ascontiguousarray(w.T), GC).astype(ml_dtypes.bfloat16)
        b = inputs[f"{nm}_b"].astype(f)[row_sel]  # (1024,)
        ins[f"{nm}_b"] = np.ascontiguousarray(b.reshape(8, 128).T).astype(f)  # [128, 8]

    h2t = inputs["h2t_w"].astype(f)
    ins["h2tT_r0"] = _tile_kT(np.ascontiguousarray(h2t[:, 0:H].T), HC).astype(ml_dtypes.bfloat16)
    ins["h2tT_r1"] = _tile_kT(np.ascontiguousarray(h2t[:, H:].T), HC).astype(ml_dtypes.bfloat16)
    ins["h2t_b"] = inputs["h2t_b"].astype(f).reshape(K, 1)

    trans = inputs["transitions"].astype(f)
    ins["transT"] = np.ascontiguousarray(trans.T)
    ins["transEnd"] = np.ascontiguousarray(trans[END_IDX].reshape(K, 1))
    a0 = np.full((K, 1), 0.0, f)
    a0[:, 0] = 0.0
    a0[START_IDX, 0] = 1.0
    ins["alpha0"] = a0
    ins["ident48"] = np.eye(K, dtype=f)
    return ins


# ----------------------------------------------------------------------------
# device program
# ----------------------------------------------------------------------------

def build():
    nc = bacc.Bacc("TRN2", target_bir_lowering=False, num_devices=2)

    def din(name, shape, dt=bf16):
        return nc.dram_tensor(name, shape, dt, kind="ExternalInput")

    embT_d = din("embT", [128, K0C * T])
    ident_d = din("ident", [128, 128])
    whh_d = {s: din(f"whhT_{s}", [128, HC * G], fp8) for s in ("enc0", "enc1", "dec0", "dec1")}
    bias_d = {s: din(f"bias_{s}", [128, GC], f32) for s in ("enc0", "enc1", "dec0", "dec1")}
    wih0_d = {m: din(f"wih0T_{m}", [128, K0C * G]) for m in ("enc", "dec")}
    wih1o_d = {m: din(f"wih1T_own_{m}", [128, HC * G]) for m in ("enc", "dec")}
    wih1p_d = {m: din(f"wih1T_peer_{m}", [128, HC * G]) for m in ("enc", "dec")}
    e2hT_d = din("e2hT", [128, GC * 1024])
    e2cT_d = din("e2cT", [128, GC * 1024])
    e2hb_d = din("e2h_b", [128, 8], f32)
    e2cb_d = din("e2c_b", [128, 8], f32)
    h2tT_r0_d = din("h2tT_r0", [128, HC * K])
    h2tT_r1_d = din("h2tT_r1", [128, HC * K])
    h2tb_d = din("h2t_b", [K, 1], f32)
    transT_d = din("transT", [K, K], f32)
    transEnd_d = din("transEnd", [K, 1], f32)
    alpha0_d = din("alpha0", [K, 1], f32)
    ident48_d = din("ident48", [K, K], f32)

    feats_out = nc.dram_tensor("feats", [K, T], f32, kind="ExternalOutput")
    lnS_out = nc.dram_tensor("lnS", [1, T], f32, kind="ExternalOutput")
    zfin_out = nc.dram_tensor("zfin", [1, 1], f32, kind="ExternalOutput")

    # internal DRAM
    xp_a = nc.dram_tensor("xp_a", [128, GC * T], bf16)  # enc0 / enc1 / dec1
    xp_b = nc.dram_tensor("xp_b", [128, GC * T], bf16)  # dec0
    hs_ag_in = nc.dram_tensor("hs_ag_in", [128, HC * (T + 1)], bf16)
    hs_ag_out = nc.dram_tensor("hs_ag_out", [256, HC * (T + 1)], bf16)
    fin_ag_in = nc.dram_tensor("fin_ag_in", [128, 16], f32)
    fin_ag_out = nc.dram_tensor("fin_ag_out", [256, 16], f32)

    RG = [[0, 1]]

    with TileContext(nc) as tc:
        with (
            tc.tile_pool(name="pw", bufs=1) as pw,  # persistent weights/state
            tc.tile_pool(name="slab", bufs=1) as slab_pool,  # wih0 scratch 4.5MB
            tc.tile_pool(name="slab1", bufs=1) as slab1_pool,  # wih1 own / whh fp8
            tc.tile_pool(name="slab2", bufs=1) as slab2_pool,  # wih1 peer / e2h
            tc.tile_pool(name="hs", bufs=2) as hs_pool,
            tc.tile_pool(name="peer", bufs=1) as peer_pool,
            tc.tile_pool(name="xpw", bufs=2) as xpw_pool,
        ):
            # PSUM pools are scoped manually: the scan/xp pools release their
            # banks before the CRF pool (8 banks for 8 interleaved chains).
            psx_cm = tc.tile_pool(name="psx", bufs=2, space="PSUM")  # xp matmuls
            psx_pool = psx_cm.__enter__()
            pss_cm = tc.tile_pool(name="pss", bufs=1, space="PSUM")  # scan, 4 tags
            pss_pool = pss_cm.__enter__()
            # ---- persistent loads
            bias = {}
            for s in ("enc0", "enc1", "dec0", "dec1"):
                bias[s] = pw.tile([128, GC], f32, name=f"bias_{s}")
                nc.sync.dma_start(out=bias[s], in_=bias_d[s][:, :])
            ident = pw.tile([128, 128], bf16, name="ident")
            nc.sync.dma_start(out=ident, in_=ident_d[:, :])

            # ---- xp matmul helper: out_dram[:, mc*T + t] over given k-slabs
            def xp_matmul(out_dram, slabs, bias_tile):
                """slabs: list of (sbuf_slab, nk, rhs_fn) triples contracting
                consecutive k-ranges; rhs_fn(kc, t0, n) -> AP [128, n] moving.
                Moving chunks are 256 wide so interleaved xp matmuls never
                block the scan's latency-critical matvec stream for long."""
                NT = 256
                for tb in range(T // NT):
                    t0 = tb * NT
                    for mc in range(GC):
                        ps = psx_pool.tile([128, NT], f32, tag="psx", name=f"psx_{tb}_{mc}")
                        first = True
                        for slab, nk, rhs_fn in slabs:
                            for kc in range(nk):
                                nc.tensor.matmul(
                                    ps,
                                    slab[:, kc * G + mc * 128 : kc * G + (mc + 1) * 128],
                                    rhs_fn(kc, t0, NT),
                                    start=first,
                                    stop=(slab is slabs[-1][0]) and kc == nk - 1,
                                )
                                first = False
                        st = xpw_pool.tile([128, NT], bf16, tag="xstage", name=f"xst_{tb}_{mc}")
                        nc.vector.tensor_scalar(
                            out=st, in0=ps, scalar1=bias_tile[:, mc : mc + 1],
                            scalar2=None, op0=ALU.add,
                        )
                        nc.sync.dma_start(
                            out=out_dram[:, mc * T + t0 : mc * T + t0 + NT], in_=st
                        )

            # ---- P0: layer-0 xp for enc and dec (embT and wih0 streamed
            # in windows; weight window per (tb, mc): [128, K0C, 128])
            embr = embT_d[:, :].rearrange("p (k t) -> p k t", k=K0C)
            NT = 256
            for model, out_dram in (("enc", xp_a), ("dec", xp_b)):
                w0r = wih0_d[model][:, :].rearrange("p (k m) -> p k m", k=K0C)
                for tb in range(T // NT):
                    t0 = tb * NT
                    ew = xpw_pool.tile([128, K0C, NT], bf16, tag="win", name=f"ew_{model}_{tb}")
                    nc.sync.dma_start(out=ew, in_=embr[:, :, t0 : t0 + NT])
                    for mc in range(GC):
                        ww = xpw_pool.tile(
                            [128, K0C, 128], bf16, tag="wwin", name=f"ww_{model}_{tb}_{mc}"
                        )
                        nc.sync.dma_start(
                            out=ww, in_=w0r[:, :, mc * 128 : (mc + 1) * 128]
                        )
                        ps = psx_pool.tile([128, NT], f32, tag="psx", name=f"psx0_{model}_{tb}_{mc}")
                        for kc in range(K0C):
                            nc.tensor.matmul(
                                ps, ww[:, kc, :], ew[:, kc, :],
                                start=(kc == 0), stop=(kc == K0C - 1),
                            )
                        st = xpw_pool.tile([128, NT], bf16, tag="xstage", name=f"x0_{model}_{tb}_{mc}")
                        nc.vector.tensor_scalar(
                            out=st, in0=ps, scalar1=bias[f"{model}0"][:, mc : mc + 1],
                            scalar2=None, op0=ALU.add,
                        )
                        nc.sync.dma_start(
                            out=out_dram[:, mc * T + t0 : mc * T + t0 + NT], in_=st
                        )

            # ---- scan helper
            def scan(s, xp_dram, Hs, c, h0_src=None, c0_src=None):
                """Run one LSTM direction scan. Hs: [128, HC*(T+1)] bf16 tile
                (time history, written once per step); c: [128, HC] f32 tile.
                The per-step matvec reads a STATIC fp8 h_cur tile so every
                matmul access pattern is static (PE pipelines at full rate)."""
                W8 = slab1_pool.tile([128, HC * G], fp8, tag="whh", name=f"whh_{s}")
                nc.sync.dma_start(out=W8, in_=whh_d[s][:, :])
                h_cur = pw.tile([128, HC], fp8, tag="hcur", name=f"hcur_{s}")
                if h0_src is None:
                    nc.vector.memset(Hs[:, 0:HC], 0.0)
                    nc.vector.memset(h_cur, 0.0)
                    nc.vector.memset(c, 0.0)
                else:
                    nc.vector.tensor_copy(Hs[:, 0:HC], h0_src)
                    nc.vector.tensor_copy(h_cur, h0_src)
                    nc.vector.tensor_copy(c, c0_src)
                sgg = pw.tile([128, HC], f32, tag="sgg", name=f"sgg_{s}")
                sgi = pw.tile([128, HC], f32, tag="sgi", name=f"sgi_{s}")
                sgf = pw.tile([128, HC], f32, tag="sgf", name=f"sgf_{s}")
                sgo = pw.tile([128, HC], f32, tag="sgo", name=f"sgo_{s}")
                ut = pw.tile([128, HC], f32, tag="ut", name=f"ut_{s}")
                mt = pw.tile([128, HC], f32, tag="mt", name=f"mt_{s}")
                tnc = pw.tile([128, HC], f32, tag="tnc", name=f"tnc_{s}")
                # gate-group order: g first (tanh(g) heads the update chain),
                # o last (only tanh(c)*sig(o) trails its activation); each
                # group gets its own PSUM bank so its activation can read
                # while the PE still accumulates later groups.
                GROUPS = [(12, sgg, AF.Tanh), (0, sgi, AF.Sigmoid),
                          (4, sgf, AF.Sigmoid), (8, sgo, AF.Sigmoid)]
                for w in range(T // CH):
                    t0 = w * CH
                    xw = xpw_pool.tile([128, GC, CH], bf16, tag="win", name=f"xw_{s}_{w}")
                    nc.sync.dma_start(
                        out=xw,
                        in_=xp_dram[:, :].rearrange("p (g t) -> p g t", g=GC)[
                            :, :, t0 : t0 + CH
                        ],
                    )
                    with tc.For_i(0, CH // U) as iv:
                        for u in range(U):
                            pst = {}
                            # xp injection first: independent of h, runs in the
                            # PE-idle window of the previous step's pointwise
                            for gi, (mc0, _, _) in enumerate(GROUPS):
                                pst[mc0] = pss_pool.tile(
                                    [128, HC], f32, tag=f"ps{gi}", name=f"ps{gi}_{s}_{u}"
                                )
                                nc.tensor.matmul(
                                    pst[mc0], ident,
                                    xw[:, mc0 : mc0 + HC, ds(U * iv + u, 1)],
                                    start=True, stop=False,
                                )
                            for mc0, _, _ in GROUPS:
                                for j in range(HC):
                                    mc = mc0 + j
                                    for kc in range(HC):
                                        nc.tensor.matmul(
                                            pst[mc0][:, j : j + 1],
                                            W8[:, kc * G + mc * 128 : kc * G + (mc + 1) * 128],
                                            h_cur[:, kc : kc + 1],
                                            start=False,
                                            stop=(j == HC - 1) and (kc == HC - 1),
                                            skip_group_check=True,
                                        )
                            for mc0, st, fn in GROUPS:
                                nc.scalar.activation(st, pst[mc0], fn)
                            nc.vector.tensor_tensor(out=ut, in0=sgi, in1=sgg, op=ALU.mult)
                            nc.vector.tensor_tensor(out=mt, in0=sgf, in1=c, op=ALU.mult)
                            nc.vector.tensor_tensor(out=c, in0=mt, in1=ut, op=ALU.add)
                            nc.scalar.activation(tnc, c, AF.Tanh)
                            hbase = HC * t0 + HC * U * iv + HC * u
                            nc.vector.tensor_tensor(
                                out=h_cur, in0=sgo, in1=tnc, op=ALU.mult
                            )
                            nc.vector.tensor_copy(Hs[:, ds(hbase + HC, HC)], h_cur)

            # ---- AllGather of an Hs buffer; returns peer tile (peer's order).
            def exchange_hs(Hs, tagsuffix):
                nc.sync.dma_start(out=hs_ag_in[:, :], in_=Hs)
                nc.gpsimd.collective_compute(
                    "AllGather", ALU.bypass,
                    ins=[hs_ag_in[:, :]], outs=[hs_ag_out[:, :]], replica_groups=RG,
                )
                peer = peer_pool.tile(
                    [128, HC * (T + 1)], bf16, tag="peer", name=f"peer_{tagsuffix}"
                )
                CW = 1026  # 8 chunks cover HC*(T+1) = 8196 (last chunk 1014)
                for ci in range(8):
                    lo = ci * CW
                    hi = min(HC * (T + 1), lo + CW)
                    n = hi - lo
                    b0 = peer_pool.tile([128, CW], bf16, tag="pb0", name=f"pb0_{tagsuffix}_{ci}")
                    b1 = peer_pool.tile([128, CW], bf16, tag="pb1", name=f"pb1_{tagsuffix}_{ci}")
                    nc.sync.dma_start(out=b0[:, :n], in_=hs_ag_out[0:128, lo:hi])
                    nc.sync.dma_start(out=b1[:, :n], in_=hs_ag_out[128:256, lo:hi])
                    pf = peer_pool.tile([128, CW], f32, tag="pf", name=f"pf_{tagsuffix}_{ci}")
                    nc.vector.tensor_tensor(out=pf[:, :n], in0=b0[:, :n], in1=b1[:, :n], op=ALU.add)
                    nc.vector.tensor_tensor(out=pf[:, :n], in0=pf[:, :n], in1=Hs[:, lo:hi], op=ALU.subtract)
                    nc.vector.tensor_copy(peer[:, lo:hi], pf[:, :n])
                return peer

            # reversed-read AP into peer Hs outputs
            def peer_rev_ap(peer, kc, t0, n):
                return peer[:, :].rearrange("p (t c) -> p t c", c=HC)[
                    :, T - t0 : T - t0 - n : -1, kc
                ]

            # ---- ENC pipeline
            Hs_e0 = hs_pool.tile([128, HC * (T + 1)], bf16, tag="Hs", name="Hs_enc0")
            c_e0 = pw.tile([128, HC], f32, name="c_enc0")
            scan("enc0", xp_a, Hs_e0, c_e0)

            peer_e0 = exchange_hs(Hs_e0, "enc")
            own1 = slab1_pool.tile([128, HC * G], bf16, tag="slab1", name="w1o_enc")
            nc.sync.dma_start(out=own1, in_=wih1o_d["enc"][:, :])
            peer1 = slab2_pool.tile([128, HC * G], bf16, tag="slab2", name="w1p_enc")
            nc.sync.dma_start(out=peer1, in_=wih1p_d["enc"][:, :])
            xp_matmul(
                xp_a,
                [
                    (own1, HC, lambda kc, t0, n: Hs_e0[:, :].rearrange(
                        "p (t c) -> p t c", c=HC)[:, t0 + 1 : t0 + 1 + n, kc]),
                    (peer1, HC, lambda kc, t0, n: peer_rev_ap(peer_e0, kc, t0, n)),
                ],
                bias["enc1"],
            )
            Hs_e1 = hs_pool.tile([128, HC * (T + 1)], bf16, tag="Hs", name="Hs_enc1")
            c_e1 = pw.tile([128, HC], f32, name="c_enc1")
            scan("enc1", xp_a, Hs_e1, c_e1)

            # ---- finals AG + init-state matvecs
            fin = pw.tile([128, 16], f32, name="fin")
            nc.vector.tensor_copy(fin[:, 0:4], Hs_e0[:, HC * T : HC * T + 4])
            nc.vector.tensor_copy(fin[:, 4:8], Hs_e1[:, HC * T : HC * T + 4])
            nc.vector.tensor_copy(fin[:, 8:12], c_e0)
            nc.vector.tensor_copy(fin[:, 12:16], c_e1)
            nc.sync.dma_start(out=fin_ag_in[:, :], in_=fin)
            nc.gpsimd.collective_compute(
                "AllGather", ALU.bypass,
                ins=[fin_ag_in[:, :]], outs=[fin_ag_out[:, :]], replica_groups=RG,
            )
            enc_all = pw.tile([128, 32], f32, name="enc_all")
            nc.sync.dma_start(out=enc_all[:, 0:16], in_=fin_ag_out[0:128, :])
            nc.sync.dma_start(out=enc_all[:, 16:32], in_=fin_ag_out[128:256, :])

            e2hb = pw.tile([128, 8], f32, name="e2hb")
            nc.sync.dma_start(out=e2hb, in_=e2hb_d[:, :])
            e2cb = pw.tile([128, 8], f32, name="e2cb")
            nc.sync.dma_start(out=e2cb, in_=e2cb_d[:, :])
            enc_all_bf = pw.tile([128, 32], bf16, name="enc_all_bf")
            nc.vector.tensor_copy(enc_all_bf, enc_all)
            hcols = list(range(0, 8)) + list(range(16, 24))
            ccols = list(range(8, 16)) + list(range(24, 32))
            init_h = pw.tile([128, 8], f32, name="init_h")
            init_c = pw.tile([128, 8], f32, name="init_c")
            for (wd, cols, bt, out_t) in (
                (e2hT_d, hcols, e2hb, init_h),
                (e2cT_d, ccols, e2cb, init_c),
            ):
                wr = wd[:, :].rearrange("p (k m) -> p k m", k=GC)
                ps = psx_pool.tile([128, 8], f32, tag="psx", name=f"ps_init_{out_t.name}")
                for m in range(8):
                    eww = xpw_pool.tile(
                        [128, GC, 128], bf16, tag="wwin", name=f"e2w_{out_t.name}_{m}"
                    )
                    nc.sync.dma_start(out=eww, in_=wr[:, :, m * 128 : (m + 1) * 128])
                    for kc in range(GC):
                        nc.tensor.matmul(
                            ps[:, m : m + 1],
                            eww[:, kc, :],
                            enc_all_bf[:, cols[kc] : cols[kc] + 1],
                            start=(kc == 0),
                            stop=(kc == GC - 1),
                        )
                nc.vector.tensor_tensor(out=out_t, in0=ps, in1=bt, op=ALU.add)
            init_h_bf = pw.tile([128, 8], bf16, name="init_h_bf")
            nc.vector.tensor_copy(init_h_bf, init_h)

            # ---- DEC pipeline
            Hs_d0 = hs_pool.tile([128, HC * (T + 1)], bf16, tag="Hs", name="Hs_dec0")
            c_d0 = pw.tile([128, HC], f32, name="c_dec0")
            scan("dec0", xp_b, Hs_d0, c_d0, init_h_bf[:, 0:4], init_c[:, 0:4])

            peer_d0 = exchange_hs(Hs_d0, "dec")
            own1d = slab1_pool.tile([128, HC * G], bf16, tag="slab1", name="w1o_dec")
            nc.sync.dma_start(out=own1d, in_=wih1o_d["dec"][:, :])
            peer1d = slab2_pool.tile([128, HC * G], bf16, tag="slab2", name="w1p_dec")
            nc.sync.dma_start(out=peer1d, in_=wih1p_d["dec"][:, :])
            xp_matmul(
                xp_a,
                [
                    (own1d, HC, lambda kc, t0, n: Hs_d0[:, :].rearrange(
                        "p (t c) -> p t c", c=HC)[:, t0 + 1 : t0 + 1 + n, kc]),
                    (peer1d, HC, lambda kc, t0, n: peer_rev_ap(peer_d0, kc, t0, n)),
                ],
                bias["dec1"],
            )
            Hs_d1 = hs_pool.tile([128, HC * (T + 1)], bf16, tag="Hs", name="Hs_dec1")
            c_d1 = pw.tile([128, HC], f32, name="c_dec1")
            scan("dec1", xp_a, Hs_d1, c_d1, init_h_bf[:, 4:8], init_c[:, 4:8])

            # ---- feats: AllGather dec-L1 outputs; each core computes the
            # full feats identically.
            nc.sync.dma_start(out=hs_ag_in[:, :], in_=Hs_d1)
            nc.gpsimd.collective_compute(
                "AllGather", ALU.bypass,
                ins=[hs_ag_in[:, :]], outs=[hs_ag_out[:, :]], replica_groups=RG,
            )
            r0b = peer_pool.tile([128, HC * (T + 1)], bf16, tag="peer", name="d1_r0")
            nc.sync.dma_start(out=r0b, in_=hs_ag_out[0:128, :])
            r1b = peer_pool.tile([128, HC * (T + 1)], bf16, tag="peerb", name="d1_r1")
            nc.sync.dma_start(out=r1b, in_=hs_ag_out[128:256, :])
            h2tT0 = pw.tile([128, HC * K], bf16, name="h2tT0")
            nc.sync.dma_start(out=h2tT0, in_=h2tT_r0_d[:, :])
            h2tT1 = pw.tile([128, HC * K], bf16, name="h2tT1")
            nc.sync.dma_start(out=h2tT1, in_=h2tT_r1_d[:, :])
            feats = pw.tile([K, T], f32, name="feats")
            NT = 512
            r0r = r0b[:, :].rearrange("p (t c) -> p t c", c=HC)
            r1r = r1b[:, :].rearrange("p (t c) -> p t c", c=HC)
            for tb in range(T // NT):
                t0 = tb * NT
                ps = psx_pool.tile([K, NT], f32, tag="psx", name=f"psf_{tb}")
                for kc in range(HC):
                    nc.tensor.matmul(
                        ps, h2tT0[:, kc * K : (kc + 1) * K],
                        r0r[:, t0 + 1 : t0 + 1 + NT, kc],
                        start=(kc == 0), stop=False,
                    )
                for kc in range(HC):
                    nc.tensor.matmul(
                        ps, h2tT1[:, kc * K : (kc + 1) * K],
                        r1r[:, T - t0 : T - t0 - NT : -1, kc],
                        start=False, stop=(kc == HC - 1),
                    )
                nc.vector.tensor_copy(feats[:, t0 : t0 + NT], ps)
            h2tb = pw.tile([K, 1], f32, name="h2tb")
            nc.sync.dma_start(out=h2tb, in_=h2tb_d[:, :])
            nc.vector.tensor_scalar(
                out=feats, in0=feats, scalar1=h2tb, scalar2=None, op0=ALU.add
            )
            nc.sync.dma_start(out=feats_out[:, :], in_=feats)

            # ---- CRF forward (linear domain, 8 interleaved transfer-matrix
            # chains of T/8 steps each; per chain-step one fixed-stationary
            # matmul P @ M and one per-row scale by exp(feats_t) on the DVE,
            # renormalized every RN steps; the chains hide each other's
            # PE<->DVE round-trip latency).
            pss_cm.__exit__(None, None, None)
            psx_cm.__exit__(None, None, None)
            psc_cm = tc.tile_pool(name="psc", bufs=1, space="PSUM")
            psc_pool = psc_cm.__enter__()

            NCH = 8
            BL = T // NCH  # 256
            RN = 4

            expF = pw.tile([K, T], f32, name="expF")
            nc.scalar.activation(expF, feats, AF.Exp)
            transT_sb = pw.tile([K, K], f32, name="transT_sb")
            nc.sync.dma_start(out=transT_sb, in_=transT_d[:, :])
            PexpT = pw.tile([K, K], f32, name="PexpT")
            nc.scalar.activation(PexpT, transT_sb, AF.Exp)
            transEnd_sb = pw.tile([K, 1], f32, name="transEnd_sb")
            nc.sync.dma_start(out=transEnd_sb, in_=transEnd_d[:, :])
            expTE = pw.tile([K, 1], f32, name="expTE")
            nc.scalar.activation(expTE, transEnd_sb, AF.Exp)
            ident48_sb = pw.tile([K, K], f32, name="ident48_sb")
            nc.sync.dma_start(out=ident48_sb, in_=ident48_d[:, :])
            ones48 = pw.tile([K, K], f32, name="ones48")
            nc.vector.memset(ones48, 1.0)
            lnS_sb = pw.tile([1, T], f32, name="lnS_sb")
            nc.vector.memset(lnS_sb, 0.0)
            rs = pw.tile([K, NCH], f32, name="rs_crf")
            Mch = []
            for cch in range(NCH):
                Mc = pw.tile([K, K], f32, name=f"Mc_{cch}")
                nc.vector.tensor_copy(Mc, ident48_sb)
                Mch.append(Mc)

            for j in range(BL):
                for cch in range(NCH):
                    t = cch * BL + j
                    ps = psc_pool.tile([K, K], f32, tag=f"c{cch}", name=f"crf_{cch}_{j}")
                    nc.tensor.matmul(ps, PexpT, Mch[cch], start=True, stop=True)
                    if cch % 2 == 0:
                        nc.vector.tensor_scalar(
                            out=Mch[cch], in0=ps, scalar1=expF[:, t : t + 1],
                            scalar2=None, op0=ALU.mult,
                        )
                    else:
                        nc.scalar.activation(
                            Mch[cch], ps, AF.Copy, scale=expF[:, t : t + 1]
                        )
                if j % RN == RN - 1:
                    for cch in range(NCH):
                        ps2 = psc_pool.tile([K, K], f32, tag=f"c{cch}", name=f"crfs_{cch}_{j}")
                        nc.tensor.matmul(
                            ps2[:, 0:1], ones48, Mch[cch][:, 0:1], start=True, stop=True
                        )
                        slot = cch * (BL // RN) + j // RN
                        nc.scalar.activation(
                            lnS_sb[0:1, slot : slot + 1], ps2[0:1, 0:1], AF.Ln
                        )
                        nc.vector.reciprocal(rs[:, cch : cch + 1], ps2[:, 0:1])
                        nc.vector.tensor_scalar(
                            out=Mch[cch], in0=Mch[cch], scalar1=rs[:, cch : cch + 1],
                            scalar2=None, op0=ALU.mult,
                        )

            # combine: alpha = M_7 @ ... @ M_0 @ alpha0 (transpose each M on
            # the PE so it can serve as a stationary operand)
            av = pw.tile([K, 1], f32, name="av")
            nc.sync.dma_start(out=av, in_=alpha0_d[:, :])
            MT = pw.tile([K, K], f32, name="MT_comb")
            for cch in range(NCH):
                pst = psc_pool.tile([K, K], f32, tag=f"c{cch}", name=f"ctr_{cch}")
                nc.tensor.transpose(pst, Mch[cch], ident48_sb)
                nc.vector.tensor_copy(MT, pst)
                psv = psc_pool.tile([K, K], f32, tag=f"c{(cch + 1) % NCH}", name=f"cmb_{cch}")
                nc.tensor.matmul(psv[:, 0:1], MT, av, start=True, stop=True)
                nc.vector.tensor_copy(av, psv[:, 0:1])
            psZ = psc_pool.tile([K, K], f32, tag="c0", name="psZ")
            nc.tensor.matmul(psZ[0:1, 0:1], av, expTE, start=True, stop=True)
            zf = pw.tile([1, 1], f32, name="zf")
            nc.scalar.activation(zf, psZ[0:1, 0:1], AF.Ln)
            nc.sync.dma_start(out=zfin_out[:, :], in_=zf)
            nc.sync.dma_start(out=lnS_out[:, :], in_=lnS_sb)
            psc_cm.__exit__(None, None, None)
    nc.compile()
    return nc


# ----------------------------------------------------------------------------
# entry point
# ----------------------------------------------------------------------------

def _postprocess(r0, inputs):
    feats = r0["feats"].astype(np.float64)  # [K, T]
    lnS = r0["lnS"].astype(np.float64)[0]
    zfin = float(r0["zfin"][0, 0])
    Z = float(lnS.sum() + zfin)

    tags = np.asarray(inputs["tags"]).astype(np.int64)
    trans = np.asarray(inputs["transitions"]).astype(np.float64)
    ext = np.concatenate([[START_IDX], tags])
    score = trans[ext[1:], ext[:-1]].sum() + feats[tags, np.arange(T)].sum()
    score += trans[END_IDX, tags[-1]]
    return np.float32(Z - score)


def kernel(**inputs) -> np.ndarray:
    if "nc" not in _CACHE:
        _CACHE["nc"] = build()
    nc = _CACHE["nc"]
    in_maps = [_prep_core(inputs, 0), _prep_core(inputs, 1)]
    res = run_bass_kernel_spmd(nc, in_maps, [0, 1])
    return _postprocess(res.results[0], inputs)
